# revision 1
# baseline (speedup 1.0000x reference)
"""Trainium2 Bass kernel for ColorMatchingLoss (chamfer loss over YUV-sampled grids).

Math: x, y are [N=12288, B=64] grids sampled from YUV-converted images.
  d[i,j] = clip(|x_i - y_j|^2, 1e-5, 1e5)/64 + 1 - <x_i, y_j>/((|x_i|+eps)(|y_j|+eps))
  out = max(mean_i min_j d, mean_j min_i d)
For this input distribution |x_i - y_j|^2 in [2.9, 46.2], so the clip never binds.

Device strategy (8 cores, shard the 12288 x-rows, each core computes its
1536-row block of the distance matrix against the full y):
  Per 128x512 tile, PSUM accumulates d' = d - 1 via two fp16 matmuls:
    MM_e (K=66): [-x/32 ; xsq/64 ; 1]^T @ [y ; 1 ; ysq/64]  = |x-y|^2/64
    MM_t (K=64): [-(rx*x)]^T @ [ry*y]                       = -(cos-part)
  ScalarE copies PSUM->SBUF fp16; VectorE folds row-mins via a tt-min tree
  (2x f16 mode) and col-min partials via tt-min accumulate into a
  [128, 12288] running tile. Host finishes: col-min partition/core reduce,
  means, +1, max.

Measured on HW (axon trn2): relative error 1.8e-07 vs reference; steady-state
body ~172-190us/core/exec (VectorE-bound at its 2x_1P floor: ~2.37us per
128x2048 tile-group = row-min tree 1.25us + col-min accumulate 1.13us, with
TensorE ~1.7us and ScalarE ~1.85us per group hidden underneath).
Notes: native TENSOR_TENSOR_REDUCE (fused tt+min-reduce) crashes the DVE on
this deployment (NRT_EXEC_UNIT_UNRECOVERABLE) in every param combo tested;
gpsimd dma accum_op=min fails walrus birverifier. Both would otherwise cut
the VectorE bottleneck.
"""

import os

import numpy as np

N = 12288          # total grid rows (2 channels * 6144 samples)
B = 64             # feature dim (batch)
NCORES = 8
R = N // NCORES    # 1536 rows per core
MCHUNKS = R // 128  # 12
GROUP_COLS = 2048  # 4 psum banks per epilogue group
NGROUPS = N // GROUP_COLS  # 6

YUV_UV = np.array([[-0.14714119, -0.28886916, 0.43601035],
                   [0.61497538, -0.51496512, -0.10001026]], dtype=np.float32)
EPS = 1e-16

_compiled = None


def _build_bass(colmin_dma_ranges=(), act_split_cols=0, mm_n=512, loop_mult=1,
                s_bufs=3, sc_bufs=2, racc_bufs=2, rowmin_ts=True,
                epi="full", explicit_ldw=False):
    """colmin_dma_ranges: set of group indices whose col-min accumulation
    runs on gpsimd DMA (CCE min) instead of VectorE.
    act_split_cols: leading columns of each group copied PSUM->SBUF by
    VectorE instead of ScalarE (load balance knob)."""
    from contextlib import ExitStack

    import concourse.bacc as bacc
    import concourse.bass as bass
    import concourse.tile as tile
    from concourse import mybir

    f16 = mybir.dt.float16
    f32 = mybir.dt.float32
    MIN = mybir.AluOpType.min

    nc = bacc.Bacc("TRN2", target_bir_lowering=False, debug=False,
                   num_devices=NCORES)

    lhsT_e_d = nc.dram_tensor("lhsT_e", [66, R], f16, kind="ExternalInput")
    lhsT_t_d = nc.dram_tensor("lhsT_t", [64, R], f16, kind="ExternalInput")
    rhs_e_d = nc.dram_tensor("rhs_e", [66, N], f16, kind="ExternalInput")
    rhs_t_d = nc.dram_tensor("rhs_t", [64, N], f16, kind="ExternalInput")
    rowmins_d = nc.dram_tensor("rowmins", [128, MCHUNKS], f32,
                               kind="ExternalOutput")
    colmins_d = nc.dram_tensor("colmins", [128, N], f16, kind="ExternalOutput")

    with tile.TileContext(nc) as tc, ExitStack() as ctx:
        consts = ctx.enter_context(tc.tile_pool(name="consts", bufs=1))
        spool = ctx.enter_context(tc.tile_pool(name="spool", bufs=s_bufs))
        scpool = ctx.enter_context(tc.tile_pool(name="scpool", bufs=sc_bufs))
        raccpool = ctx.enter_context(tc.tile_pool(name="racc", bufs=racc_bufs))
        pspool = ctx.enter_context(
            tc.tile_pool(name="psum", bufs=2, space=bass.MemorySpace.PSUM))

        lhsT_e = consts.tile([66, R], f16)
        lhsT_t = consts.tile([64, R], f16)
        rhs_e = consts.tile([66, N], f16)
        rhs_t = consts.tile([64, N], f16)
        colacc = consts.tile([128, N], f16)
        rowmins = consts.tile([128, MCHUNKS], f32)
        rowparts = consts.tile([128, MCHUNKS * NGROUPS], f32)

        nc.sync.dma_start(lhsT_e[:], lhsT_e_d.ap())
        nc.sync.dma_start(lhsT_t[:], lhsT_t_d.ap())
        # chunk the big rhs DMAs so the first groups can start early
        for c in range(NGROUPS):
            cs = slice(c * GROUP_COLS, (c + 1) * GROUP_COLS)
            nc.sync.dma_start(rhs_e[:, cs], rhs_e_d.ap()[:, cs])
            nc.sync.dma_start(rhs_t[:, cs], rhs_t_d.ap()[:, cs])

        if colmin_dma_ranges:
            # DMA accum path needs colacc pre-set to +inf-ish
            for c in range(NGROUPS):
                if c in colmin_dma_ranges:
                    nc.gpsimd.memset(
                        colacc[:, c * GROUP_COLS:(c + 1) * GROUP_COLS], 60000.0)

        if loop_mult == 0:  # null kernel for overhead calibration
            nc.gpsimd.memset(rowmins[:], 0.0)
            nc.gpsimd.memset(colacc[:], 0.0)
            nc.sync.dma_start(colmins_d.ap(), colacc[:])

        for mi in range(MCHUNKS * loop_mult):
            m = mi % MCHUNKS
            ms = slice(m * 128, (m + 1) * 128)
            racc = (None if rowmin_ts else
                    raccpool.tile([128, 512], f16, tag="racc"))
            for g in range(NGROUPS):
                n0 = g * GROUP_COLS
                ps = pspool.tile([128, GROUP_COLS], f32)
                if explicit_ldw:
                    nc.tensor.ldweights(lhsT_e[:, ms])
                for k in range(GROUP_COLS // mm_n):
                    ks = slice(k * mm_n, (k + 1) * mm_n)
                    ns = slice(n0 + k * mm_n, n0 + (k + 1) * mm_n)
                    nc.tensor.matmul(ps[:, ks], lhsT_e[:, ms], rhs_e[:, ns],
                                     start=True, stop=False)
                if explicit_ldw:
                    nc.tensor.ldweights(lhsT_t[:, ms])
                for k in range(GROUP_COLS // mm_n):
                    ks = slice(k * mm_n, (k + 1) * mm_n)
                    ns = slice(n0 + k * mm_n, n0 + (k + 1) * mm_n)
                    nc.tensor.matmul(ps[:, ks], lhsT_t[:, ms], rhs_t[:, ns],
                                     start=False, stop=True)

                if epi == "none":
                    continue
                s = spool.tile([128, GROUP_COLS], f16, tag="s")
                if act_split_cols:
                    nc.vector.tensor_copy(s[:, 0:act_split_cols],
                                          ps[:, 0:act_split_cols])
                    nc.scalar.activation(s[:, act_split_cols:],
                                         ps[:, act_split_cols:],
                                         mybir.ActivationFunctionType.Copy)
                else:
                    nc.scalar.activation(s[:], ps[:],
                                         mybir.ActivationFunctionType.Copy)

                cs = slice(n0, n0 + GROUP_COLS)
                rp = rowparts[:, m * NGROUPS + g:m * NGROUPS + g + 1]
                if epi == "act":
                    continue
                if epi == "nocol":
                    sc1 = scpool.tile([128, GROUP_COLS], f16, tag="sc1")
                    nc.vector.tensor_scalar(
                        out=sc1[:], in0=s[:], scalar1=0.0, scalar2=None,
                        op0=mybir.AluOpType.bypass, op1=MIN, accum_out=rp)
                elif rowmin_ts:
                    # single-src tensor_scalar: out = bypass(s), side output
                    # accum_out = min-reduce(out) -> row-min of the group in
                    # one 4x-mode pass. For m==0 the bypass copy doubles as
                    # the colacc initialisation.
                    BYP = mybir.AluOpType.bypass
                    if m == 0:
                        nc.vector.tensor_scalar(
                            out=colacc[:, cs], in0=s[:], scalar1=0.0,
                            scalar2=None, op0=BYP, op1=MIN, accum_out=rp)
                    else:
                        sc1 = scpool.tile([128, GROUP_COLS], f16, tag="sc1")
                        nc.vector.tensor_scalar(
                            out=sc1[:], in0=s[:], scalar1=0.0,
                            scalar2=None, op0=BYP, op1=MIN, accum_out=rp)
                        nc.vector.tensor_tensor(colacc[:, cs], colacc[:, cs],
                                                s[:], op=MIN)
                else:
                    # row-min: two tt-min tree levels into the running racc
                    sc1 = scpool.tile([128, 1024], f16, tag="sc1")
                    nc.vector.tensor_tensor(sc1[:], s[:, 0:1024],
                                            s[:, 1024:2048], op=MIN)
                    if g == 0:
                        nc.vector.tensor_tensor(racc[:], sc1[:, 0:512],
                                                sc1[:, 512:1024], op=MIN)
                    else:
                        sc2 = scpool.tile([128, 512], f16, tag="sc2")
                        nc.vector.tensor_tensor(sc2[:], sc1[:, 0:512],
                                                sc1[:, 512:1024], op=MIN)
                        nc.vector.tensor_tensor(racc[:], racc[:], sc2[:],
                                                op=MIN)

                if g in colmin_dma_ranges:
                    nc.gpsimd.dma_start(colacc[:, cs], s[:],
                                        accum_op=MIN)
                elif m == 0:
                    if not rowmin_ts:
                        nc.vector.tensor_copy(colacc[:, cs], s[:])
                elif not rowmin_ts:
                    nc.vector.tensor_tensor(colacc[:, cs], colacc[:, cs],
                                            s[:], op=MIN)
                if mi == MCHUNKS * loop_mult - 1 and epi == "full":
                    nc.sync.dma_start(colmins_d.ap()[:, cs], colacc[:, cs])

            if not rowmin_ts:
                nc.vector.tensor_reduce(rowmins[:, m:m + 1], racc[:],
                                        axis=mybir.AxisListType.X, op=MIN)

        if rowmin_ts and epi in ("full", "nocol") and loop_mult > 0:
            rpv = rowparts[:].rearrange("p (m g) -> p m g", g=NGROUPS)
            nc.vector.tensor_reduce(rowmins[:], rpv,
                                    axis=mybir.AxisListType.X, op=MIN)
        elif epi != "full" or loop_mult == 0:
            nc.gpsimd.memset(rowmins[:], 0.0)
        if epi != "full":
            nc.gpsimd.memset(colacc[:], 0.0)
            nc.sync.dma_start(colmins_d.ap(), colacc[:])
        nc.sync.dma_start(rowmins_d.ap(), rowmins[:])

    nc.compile()
    return nc


def _prepare_inputs(input_img, target_img, inds_y_input, inds_x_input,
                    inds_y_target, inds_x_target):
    input_img = np.asarray(input_img, dtype=np.float32)
    target_img = np.asarray(target_img, dtype=np.float32)
    iy_i = np.asarray(inds_y_input).astype(np.int64)
    ix_i = np.asarray(inds_x_input).astype(np.int64)
    iy_t = np.asarray(inds_y_target).astype(np.int64)
    ix_t = np.asarray(inds_x_target).astype(np.int64)

    def build_grid(img, iy, ix):
        g = (img[:, :, iy, ix] + 1.0) / 2.0          # [B,3,n]
        yuv = np.einsum('bcn,dc->bdn', g, YUV_UV)    # [B,2,n]
        return yuv.reshape(yuv.shape[0], -1).T.astype(np.float32)  # [2n,B]

    x = build_grid(input_img, iy_i, ix_i)   # [N, B]
    y = build_grid(target_img, iy_t, ix_t)  # [N, B]

    xsq = np.einsum('ij,ij->i', x, x)
    ysq = np.einsum('ij,ij->i', y, y)
    rx = 1.0 / (np.sqrt(xsq) + EPS)
    ry = 1.0 / (np.sqrt(ysq) + EPS)

    f16 = np.float16
    rhs_e = np.empty((66, N), dtype=f16)
    rhs_e[0:64] = y.T.astype(f16)
    rhs_e[64] = 1.0
    rhs_e[65] = (ysq / 64.0).astype(f16)
    rhs_t = (y * ry[:, None]).T.astype(f16)

    lhsT_e_full = np.empty((66, N), dtype=f16)
    lhsT_e_full[0:64] = (-x / 32.0).T.astype(f16)
    lhsT_e_full[64] = (xsq / 64.0).astype(f16)
    lhsT_e_full[65] = 1.0
    lhsT_t_full = (-(x * rx[:, None])).T.astype(f16)

    in_maps = []
    for c in range(NCORES):
        rs = slice(c * R, (c + 1) * R)
        in_maps.append({
            "lhsT_e": np.ascontiguousarray(lhsT_e_full[:, rs]),
            "lhsT_t": np.ascontiguousarray(lhsT_t_full[:, rs]),
            "rhs_e": rhs_e,
            "rhs_t": rhs_t,
        })
    return in_maps


def _combine(results):
    rowmin_all = np.concatenate(
        [r["rowmins"].T.reshape(-1) for r in results])        # [N]
    colmin_stack = np.stack([r["colmins"] for r in results])  # [8,128,N]
    colmin = colmin_stack.astype(np.float32).min(axis=(0, 1))  # [N]
    m1 = 1.0 + rowmin_all.mean()
    m2 = 1.0 + colmin.mean()
    return np.asarray(np.float32(max(m1, m2)))


def kernel(input_img, target_img, inds_y_input, inds_x_input,
           inds_y_target, inds_x_target):
    global _compiled
    import time

    from concourse import bass_utils

    if _compiled is None:
        _compiled = _build_bass()
    nc = _compiled

    in_maps = _prepare_inputs(input_img, target_img, inds_y_input,
                              inds_x_input, inds_y_target, inds_x_target)
    # Retry: a previously-crashed tenant can leave the NeuronCore wedged
    # (NRT_EXEC_UNIT_UNRECOVERABLE) for one execution attempt before it
    # self-clears; a fresh attempt then succeeds.
    last_err = None
    for attempt in range(4):
        try:
            res = bass_utils.run_bass_kernel_spmd(
                nc, in_maps, core_ids=list(range(NCORES)))
            return _combine(res.results)
        except Exception as e:  # noqa: BLE001
            last_err = e
            time.sleep(3.0)
    raise last_err



# revision 21
# speedup vs baseline: 1.6728x; 1.6728x over previous
"""Trainium2 Bass kernel for ColorMatchingLoss (chamfer loss over YUV-sampled grids).

Math: x, y are [N=12288, B=64] grids sampled from YUV-converted images.
  d[i,j] = clip(|x_i - y_j|^2, 1e-5, 1e5)/64 + 1 - <x_i, y_j>/((|x_i|+eps)(|y_j|+eps))
  out = max(mean_i min_j d, mean_j min_i d)
For this input distribution |x_i - y_j|^2 in [2.9, 46.2], so the clip never binds.

Device strategy (8 cores, shard the 12288 x-rows, each core computes its
1536-row block of the distance matrix against the full y):
  Per 128x512 tile, PSUM accumulates d' = d - 1 via two fp16 matmuls:
    MM_e (K=66): [-x/32 ; xsq/64 ; 1]^T @ [y ; 1 ; ysq/64]  = |x-y|^2/64
    MM_t (K=64): [-(rx*x)]^T @ [ry*y]                       = -(cos-part)
  ScalarE copies PSUM->SBUF fp16; VectorE folds row-mins via a tt-min tree
  (2x f16 mode) and col-min partials via tt-min accumulate into a
  [128, 12288] running tile. Host finishes: col-min partition/core reduce,
  means, +1, max.

Measured on HW (axon trn2): relative error 1.8e-07 vs reference; steady-state
body ~172-190us/core/exec (VectorE-bound at its 2x_1P floor: ~2.37us per
128x2048 tile-group = row-min tree 1.25us + col-min accumulate 1.13us, with
TensorE ~1.7us and ScalarE ~1.85us per group hidden underneath).
Notes: native TENSOR_TENSOR_REDUCE (fused tt+min-reduce) crashes the DVE on
this deployment (NRT_EXEC_UNIT_UNRECOVERABLE) in every param combo tested;
gpsimd dma accum_op=min fails walrus birverifier. Both would otherwise cut
the VectorE bottleneck.
"""

import os

import numpy as np

N = 12288          # total grid rows (2 channels * 6144 samples)
B = 64             # feature dim (batch)
NCORES = 8
R = N // NCORES    # 1536 rows per core
MCHUNKS = R // 128  # 12
GROUP_COLS = 2048  # 4 psum banks per epilogue group
NGROUPS = N // GROUP_COLS  # 6

YUV_UV = np.array([[-0.14714119, -0.28886916, 0.43601035],
                   [0.61497538, -0.51496512, -0.10001026]], dtype=np.float32)
EPS = 1e-16

_compiled = None


def _build_bass(colmin_dma_ranges=(), act_split_cols=0, mm_n=512, loop_mult=1,
                s_bufs=3, sc_bufs=2, racc_bufs=2, rowmin_ts=True,
                epi="full", explicit_ldw=False, chain=False,
                mm_mode="normal", pool_rowmin_gs=(), use_bf16=False):
    """colmin_dma_ranges: set of group indices whose col-min accumulation
    runs on gpsimd DMA (CCE min) instead of VectorE.
    act_split_cols: leading columns of each group copied PSUM->SBUF by
    VectorE instead of ScalarE (load balance knob)."""
    from contextlib import ExitStack

    import concourse.bacc as bacc
    import concourse.bass as bass
    import concourse.tile as tile
    from concourse import mybir

    f16 = mybir.dt.bfloat16 if use_bf16 else mybir.dt.float16
    f32 = mybir.dt.float32
    MIN = mybir.AluOpType.min

    nc = bacc.Bacc("TRN2", target_bir_lowering=False, debug=False,
                   num_devices=NCORES)

    chain_d = (nc.dram_tensor("chain", [128, MCHUNKS], f32,
                              kind="ExternalInput") if chain else None)
    lhsT_e_d = nc.dram_tensor("lhsT_e", [66, R], f16, kind="ExternalInput")
    lhsT_t_d = nc.dram_tensor("lhsT_t", [64, R], f16, kind="ExternalInput")
    rhs_e_d = nc.dram_tensor("rhs_e", [66, N], f16, kind="ExternalInput")
    rhs_t_d = nc.dram_tensor("rhs_t", [64, N], f16, kind="ExternalInput")
    rowmins_d = nc.dram_tensor("rowmins", [128, MCHUNKS], f32,
                               kind="ExternalOutput")
    colmins_d = nc.dram_tensor("colmins", [128, N], f16, kind="ExternalOutput")

    with tile.TileContext(nc) as tc, ExitStack() as ctx:
        consts = ctx.enter_context(tc.tile_pool(name="consts", bufs=1))
        spool = ctx.enter_context(tc.tile_pool(name="spool", bufs=s_bufs))
        scpool = ctx.enter_context(tc.tile_pool(name="scpool", bufs=sc_bufs))
        raccpool = ctx.enter_context(tc.tile_pool(name="racc", bufs=racc_bufs))
        pspool = ctx.enter_context(
            tc.tile_pool(name="psum", bufs=2, space=bass.MemorySpace.PSUM))

        if mm_mode == "rowtile":
            # stacked layout: rows 0-63 = e-part(y), 64-127 = t-part(yhat);
            # two row-tiled matmuls share the array concurrently.
            lhsT_c = consts.tile([128, R], f16)
            rhs_c = consts.tile([128, N], f16)
            lhsT_e = lhsT_c[0:64, :]
            lhsT_t = lhsT_c[64:128, :]
            rhs_e = rhs_c[0:64, :]
            rhs_t = rhs_c[64:128, :]
        else:
            lhsT_e = consts.tile([66, R], f16)
            lhsT_t = consts.tile([64, R], f16)
            rhs_e = consts.tile([66, N], f16)
            rhs_t = consts.tile([64, N], f16)
        colacc = consts.tile([128, N], f16)
        rowmins = consts.tile([128, MCHUNKS], f32)
        rowparts = consts.tile([128, MCHUNKS * NGROUPS], f32)

        if chain:
            chain_t = consts.tile([128, MCHUNKS], f32)
            nc.sync.dma_start(chain_t[:], chain_d.ap())
        if mm_mode == "rowtile":
            nc.sync.dma_start(lhsT_e[:], lhsT_e_d.ap()[0:64, :])
            nc.sync.dma_start(lhsT_t[:], lhsT_t_d.ap())
            for c in range(NGROUPS):
                cs = slice(c * GROUP_COLS, (c + 1) * GROUP_COLS)
                nc.sync.dma_start(rhs_e[:, cs], rhs_e_d.ap()[0:64, cs])
                nc.sync.dma_start(rhs_t[:, cs], rhs_t_d.ap()[:, cs])
        else:
            nc.sync.dma_start(lhsT_e[:], lhsT_e_d.ap())
            nc.sync.dma_start(lhsT_t[:], lhsT_t_d.ap())
            # chunk the big rhs DMAs so the first groups can start early
            for c in range(NGROUPS):
                cs = slice(c * GROUP_COLS, (c + 1) * GROUP_COLS)
                nc.sync.dma_start(rhs_e[:, cs], rhs_e_d.ap()[:, cs])
                nc.sync.dma_start(rhs_t[:, cs], rhs_t_d.ap()[:, cs])

        if colmin_dma_ranges:
            # DMA accum path needs colacc pre-set to +inf-ish
            for c in range(NGROUPS):
                if c in colmin_dma_ranges:
                    nc.gpsimd.memset(
                        colacc[:, c * GROUP_COLS:(c + 1) * GROUP_COLS], 60000.0)

        if loop_mult == 0:  # null kernel for overhead calibration
            nc.gpsimd.memset(rowmins[:], 0.0)
            nc.gpsimd.memset(colacc[:], 0.0)
            nc.sync.dma_start(colmins_d.ap(), colacc[:])

        for mi in range(MCHUNKS * loop_mult):
            m = mi % MCHUNKS
            ms = slice(m * 128, (m + 1) * 128)
            racc = (None if rowmin_ts else
                    raccpool.tile([128, 512], f16, tag="racc"))
            for g in range(NGROUPS):
                n0 = g * GROUP_COLS
                ps = pspool.tile([128, GROUP_COLS], f32)
                if mm_mode == "half":
                    for k in range(GROUP_COLS // mm_n):
                        ks = slice(k * mm_n, (k + 1) * mm_n)
                        ns = slice(n0 + k * mm_n, n0 + (k + 1) * mm_n)
                        nc.tensor.matmul(ps[:, ks], lhsT_t[:, ms],
                                         rhs_t[:, ns], start=True, stop=True)
                else:
                    if explicit_ldw:
                        nc.tensor.ldweights(lhsT_e[:, ms])
                    for k in range(GROUP_COLS // mm_n):
                        ks = slice(k * mm_n, (k + 1) * mm_n)
                        ns = slice(n0 + k * mm_n, n0 + (k + 1) * mm_n)
                        nc.tensor.matmul(ps[:, ks], lhsT_e[:, ms],
                                         rhs_e[:, ns], start=True, stop=False)
                    if explicit_ldw:
                        nc.tensor.ldweights(lhsT_t[:, ms])
                    for k in range(GROUP_COLS // mm_n):
                        ks = slice(k * mm_n, (k + 1) * mm_n)
                        ns = slice(n0 + k * mm_n, n0 + (k + 1) * mm_n)
                        nc.tensor.matmul(ps[:, ks], lhsT_t[:, ms],
                                         rhs_t[:, ns], start=False, stop=True)

                if epi == "none":
                    continue
                s = spool.tile([128, GROUP_COLS], f16, tag="s")
                if act_split_cols:
                    nc.vector.tensor_copy(s[:, 0:act_split_cols],
                                          ps[:, 0:act_split_cols])
                    nc.scalar.activation(s[:, act_split_cols:],
                                         ps[:, act_split_cols:],
                                         mybir.ActivationFunctionType.Copy)
                else:
                    nc.scalar.activation(s[:], ps[:],
                                         mybir.ActivationFunctionType.Copy)

                cs = slice(n0, n0 + GROUP_COLS)
                rp = rowparts[:, m * NGROUPS + g:m * NGROUPS + g + 1]
                if epi == "act":
                    continue
                if epi == "nocol":
                    sc1 = scpool.tile([128, GROUP_COLS], f16, tag="sc1")
                    nc.vector.tensor_scalar(
                        out=sc1[:], in0=s[:], scalar1=0.0, scalar2=None,
                        op0=mybir.AluOpType.bypass, op1=MIN, accum_out=rp)
                elif rowmin_ts:
                    # single-src tensor_scalar: out = bypass(s), side output
                    # accum_out = min-reduce(out) -> row-min of the group in
                    # one 4x-mode pass. For m==0 the bypass copy doubles as
                    # the colacc initialisation.
                    BYP = mybir.AluOpType.bypass
                    if m == 0:
                        nc.vector.tensor_scalar(
                            out=colacc[:, cs], in0=s[:], scalar1=0.0,
                            scalar2=None, op0=BYP, op1=MIN, accum_out=rp)
                    else:
                        sc1 = scpool.tile([128, GROUP_COLS], f16, tag="sc1")
                        rm_eng = (nc.gpsimd if g in pool_rowmin_gs
                                  else nc.vector)
                        rm_eng.tensor_scalar(
                            out=sc1[:], in0=s[:], scalar1=0.0,
                            scalar2=None, op0=BYP, op1=MIN, accum_out=rp)
                        nc.vector.tensor_tensor(colacc[:, cs], colacc[:, cs],
                                                s[:], op=MIN)
                else:
                    # row-min: two tt-min tree levels into the running racc
                    sc1 = scpool.tile([128, 1024], f16, tag="sc1")
                    nc.vector.tensor_tensor(sc1[:], s[:, 0:1024],
                                            s[:, 1024:2048], op=MIN)
                    if g == 0:
                        nc.vector.tensor_tensor(racc[:], sc1[:, 0:512],
                                                sc1[:, 512:1024], op=MIN)
                    else:
                        sc2 = scpool.tile([128, 512], f16, tag="sc2")
                        nc.vector.tensor_tensor(sc2[:], sc1[:, 0:512],
                                                sc1[:, 512:1024], op=MIN)
                        nc.vector.tensor_tensor(racc[:], racc[:], sc2[:],
                                                op=MIN)

                if g in colmin_dma_ranges:
                    nc.gpsimd.dma_start(colacc[:, cs], s[:],
                                        accum_op=MIN)
                elif m == 0:
                    if not rowmin_ts:
                        nc.vector.tensor_copy(colacc[:, cs], s[:])
                elif not rowmin_ts:
                    nc.vector.tensor_tensor(colacc[:, cs], colacc[:, cs],
                                            s[:], op=MIN)
                if mi == MCHUNKS * loop_mult - 1 and epi == "full":
                    nc.sync.dma_start(colmins_d.ap()[:, cs], colacc[:, cs])

            if not rowmin_ts:
                nc.vector.tensor_reduce(rowmins[:, m:m + 1], racc[:],
                                        axis=mybir.AxisListType.X, op=MIN)

        if rowmin_ts and epi in ("full", "nocol") and loop_mult > 0:
            rpv = rowparts[:].rearrange("p (m g) -> p m g", g=NGROUPS)
            nc.vector.tensor_reduce(rowmins[:], rpv,
                                    axis=mybir.AxisListType.X, op=MIN)
        elif epi != "full" or loop_mult == 0:
            nc.gpsimd.memset(rowmins[:], 0.0)
        if epi != "full":
            nc.gpsimd.memset(colacc[:], 0.0)
            nc.sync.dma_start(colmins_d.ap(), colacc[:])
        nc.sync.dma_start(rowmins_d.ap(), rowmins[:])

    nc.compile()
    return nc


def _build_v2(loop_mult=1, mm_n=512, s_bufs=3, racc_bufs=2, chain=False,
              colmin_dma=(), act_split_cols=0, epi_pair=1):
    """v2: single K=128 matmul stream per entry (cross terms only;
    PE is the half-rate bottleneck engine on this deployment).
      PSUM = -(x.y)/32 - xhat.yhat          (K=128, one column-stream)
      s    = PSUM + a_i                     (ScalarE Identity with bias AP)
      colacc[cs] = min(colacc[cs], s)       (DVE tt-min; host adds b_j + 1)
      racc_m     = min_g s_g                (DVE tt-min; y-columns are
                                             host-permuted so the 6 group
                                             slots of each column class c
                                             hold consecutive-sorted b_j;
                                             host finishes rowmin_i =
                                             min_c(racc + btilde_c) + 1)
    """
    from contextlib import ExitStack

    import concourse.bacc as bacc
    import concourse.bass as bass
    import concourse.tile as tile
    from concourse import mybir

    f16 = mybir.dt.float16
    f32 = mybir.dt.float32
    MIN = mybir.AluOpType.min

    nc = bacc.Bacc("TRN2", target_bir_lowering=False, debug=False,
                   num_devices=NCORES)

    chain_d = (nc.dram_tensor("chain", [128, MCHUNKS], f32,
                              kind="ExternalInput") if chain else None)
    lhsT_d = nc.dram_tensor("lhsT", [128, R], f16, kind="ExternalInput")
    rhs_d = nc.dram_tensor("rhs", [128, N], f16, kind="ExternalInput")
    abias_d = nc.dram_tensor("abias", [128, MCHUNKS], f32,
                             kind="ExternalInput")
    rowracc_d = nc.dram_tensor("rowracc",
                               [128, MCHUNKS * GROUP_COLS * epi_pair], f16,
                               kind="ExternalOutput")
    colmins_d = nc.dram_tensor("colmins", [128, N], f16, kind="ExternalOutput")

    with tile.TileContext(nc) as tc, ExitStack() as ctx:
        consts = ctx.enter_context(tc.tile_pool(name="consts", bufs=1))
        spool = ctx.enter_context(tc.tile_pool(name="spool", bufs=s_bufs))
        raccpool = ctx.enter_context(tc.tile_pool(name="racc",
                                                  bufs=racc_bufs))
        pspool = ctx.enter_context(
            tc.tile_pool(name="psum", bufs=2, space=bass.MemorySpace.PSUM))

        lhsT = consts.tile([128, R], f16)
        rhs = consts.tile([128, N], f16)
        abias = consts.tile([128, MCHUNKS], f32)
        colacc = consts.tile([128, N], f16)

        if chain:
            chain_t = consts.tile([128, MCHUNKS], f32)
            nc.sync.dma_start(chain_t[:], chain_d.ap())
        nc.sync.dma_start(lhsT[:], lhsT_d.ap())
        nc.sync.dma_start(abias[:], abias_d.ap())
        for c in range(NGROUPS):
            cs = slice(c * GROUP_COLS, (c + 1) * GROUP_COLS)
            nc.sync.dma_start(rhs[:, cs], rhs_d.ap()[:, cs])
        nc.gpsimd.memset(colacc[:], 60000.0)

        PW = GROUP_COLS * epi_pair           # epilogue op width
        NPAIR = NGROUPS // epi_pair
        for mi in range(MCHUNKS * loop_mult):
            m = mi % MCHUNKS
            ms = slice(m * 128, (m + 1) * 128)
            racc = raccpool.tile([128, PW], f16, tag="racc")
            for gp in range(NPAIR):
                s = spool.tile([128, PW], f16, tag="s")
                for sub in range(epi_pair):
                    g = gp * epi_pair + sub
                    n0 = g * GROUP_COLS
                    ss = slice(sub * GROUP_COLS, (sub + 1) * GROUP_COLS)
                    ps = pspool.tile([128, GROUP_COLS], f32)
                    for k in range(GROUP_COLS // mm_n):
                        ks = slice(k * mm_n, (k + 1) * mm_n)
                        ns = slice(n0 + k * mm_n, n0 + (k + 1) * mm_n)
                        nc.tensor.matmul(ps[:, ks], lhsT[:, ms], rhs[:, ns],
                                         start=True, stop=True)
                    nc.scalar.activation(
                        s[:, ss], ps[:], mybir.ActivationFunctionType.Identity,
                        bias=abias[:, m:m + 1], scale=1.0)

                cs = slice(gp * PW, (gp + 1) * PW)
                nc.vector.tensor_tensor(colacc[:, cs], colacc[:, cs],
                                        s[:], op=MIN)
                if gp == 0:
                    nc.vector.tensor_tensor(racc[:], s[:], s[:], op=MIN)
                else:
                    nc.vector.tensor_tensor(racc[:], racc[:], s[:], op=MIN)
            nc.sync.dma_start(
                rowracc_d.ap()[:, m * PW:(m + 1) * PW], racc[:])

        nc.sync.dma_start(colmins_d.ap(), colacc[:])

    nc.compile()
    return nc


_V2_META = {}


def _prepare_inputs_v2(input_img, target_img, inds_y_input, inds_x_input,
                       inds_y_target, inds_x_target, epi_pair=1):
    input_img = np.asarray(input_img, dtype=np.float32)
    target_img = np.asarray(target_img, dtype=np.float32)
    iy_i = np.asarray(inds_y_input).astype(np.int64)
    ix_i = np.asarray(inds_x_input).astype(np.int64)
    iy_t = np.asarray(inds_y_target).astype(np.int64)
    ix_t = np.asarray(inds_x_target).astype(np.int64)

    def build_grid(img, iy, ix):
        g = (img[:, :, iy, ix] + 1.0) / 2.0          # [B,3,n]
        yuv = np.einsum('bcn,dc->bdn', g, YUV_UV)    # [B,2,n]
        return yuv.reshape(yuv.shape[0], -1).T.astype(np.float32)  # [2n,B]

    x = build_grid(input_img, iy_i, ix_i)   # [N, B]
    y = build_grid(target_img, iy_t, ix_t)  # [N, B]

    xsq = np.einsum('ij,ij->i', x, x)
    ysq = np.einsum('ij,ij->i', y, y)
    rx = 1.0 / (np.sqrt(xsq) + EPS)
    ry = 1.0 / (np.sqrt(ysq) + EPS)
    a = xsq / 64.0
    b = ysq / 64.0

    # permute y columns: class c (c in [0,PW)) holds NPAIR consecutive
    # sorted ranks across the NPAIR epilogue-pair slots -> b nearly
    # constant per class (spread ~ b_range*NPAIR/N)
    pw = GROUP_COLS * epi_pair
    npair = N // pw
    order = np.argsort(b, kind="stable")
    col_at = np.empty(N, dtype=np.int64)
    for gp in range(npair):
        col_at[gp * pw:(gp + 1) * pw] = \
            order[np.arange(pw) * npair + gp]
    b_perm = b[col_at]
    btilde = b[order[np.arange(pw) * npair]]  # min of each class

    y_p = y[col_at]
    ry_p = ry[col_at]

    f16 = np.float16
    rhs = np.empty((128, N), dtype=f16)
    rhs[0:64] = y_p.T.astype(f16)
    rhs[64:128] = (y_p * ry_p[:, None]).T.astype(f16)

    lhsT_full = np.empty((128, N), dtype=f16)
    lhsT_full[0:64] = (-x / 32.0).T.astype(f16)
    lhsT_full[64:128] = (-(x * rx[:, None])).T.astype(f16)

    in_maps = []
    for c in range(NCORES):
        rs = slice(c * R, (c + 1) * R)
        abias = np.ascontiguousarray(
            a[rs].reshape(MCHUNKS, 128).T).astype(np.float32)
        in_maps.append({
            "lhsT": np.ascontiguousarray(lhsT_full[:, rs]),
            "rhs": rhs,
            "abias": abias,
        })
    _V2_META["btilde"] = btilde.astype(np.float32)
    _V2_META["b_perm"] = b_perm.astype(np.float32)
    _V2_META["pw"] = pw
    return in_maps


def _combine_v2(results):
    btilde = _V2_META["btilde"]
    b_perm = _V2_META["b_perm"]
    pw = _V2_META["pw"]
    rowmins = []
    for r in results:
        racc = r["rowracc"].astype(np.float32).reshape(128, MCHUNKS, pw)
        rowmins.append((racc + btilde[None, None, :]).min(axis=2))  # [128,12]
    m1 = 1.0 + float(np.mean(rowmins))
    colmin_stack = np.stack([r["colmins"] for r in results])  # [8,128,N]
    colmin = colmin_stack.astype(np.float32).min(axis=(0, 1))  # [N] permuted
    m2 = 1.0 + float(np.mean(colmin + b_perm))
    return np.asarray(np.float32(max(m1, m2)))


def _prepare_inputs(input_img, target_img, inds_y_input, inds_x_input,
                    inds_y_target, inds_x_target):
    input_img = np.asarray(input_img, dtype=np.float32)
    target_img = np.asarray(target_img, dtype=np.float32)
    iy_i = np.asarray(inds_y_input).astype(np.int64)
    ix_i = np.asarray(inds_x_input).astype(np.int64)
    iy_t = np.asarray(inds_y_target).astype(np.int64)
    ix_t = np.asarray(inds_x_target).astype(np.int64)

    def build_grid(img, iy, ix):
        g = (img[:, :, iy, ix] + 1.0) / 2.0          # [B,3,n]
        yuv = np.einsum('bcn,dc->bdn', g, YUV_UV)    # [B,2,n]
        return yuv.reshape(yuv.shape[0], -1).T.astype(np.float32)  # [2n,B]

    x = build_grid(input_img, iy_i, ix_i)   # [N, B]
    y = build_grid(target_img, iy_t, ix_t)  # [N, B]

    xsq = np.einsum('ij,ij->i', x, x)
    ysq = np.einsum('ij,ij->i', y, y)
    rx = 1.0 / (np.sqrt(xsq) + EPS)
    ry = 1.0 / (np.sqrt(ysq) + EPS)

    f16 = np.float16
    rhs_e = np.empty((66, N), dtype=f16)
    rhs_e[0:64] = y.T.astype(f16)
    rhs_e[64] = 1.0
    rhs_e[65] = (ysq / 64.0).astype(f16)
    rhs_t = (y * ry[:, None]).T.astype(f16)

    lhsT_e_full = np.empty((66, N), dtype=f16)
    lhsT_e_full[0:64] = (-x / 32.0).T.astype(f16)
    lhsT_e_full[64] = (xsq / 64.0).astype(f16)
    lhsT_e_full[65] = 1.0
    lhsT_t_full = (-(x * rx[:, None])).T.astype(f16)

    in_maps = []
    for c in range(NCORES):
        rs = slice(c * R, (c + 1) * R)
        in_maps.append({
            "lhsT_e": np.ascontiguousarray(lhsT_e_full[:, rs]),
            "lhsT_t": np.ascontiguousarray(lhsT_t_full[:, rs]),
            "rhs_e": rhs_e,
            "rhs_t": rhs_t,
        })
    return in_maps


def _combine(results):
    rowmin_all = np.concatenate(
        [r["rowmins"].T.reshape(-1) for r in results])        # [N]
    colmin_stack = np.stack([r["colmins"] for r in results])  # [8,128,N]
    colmin = colmin_stack.astype(np.float32).min(axis=(0, 1))  # [N]
    m1 = 1.0 + rowmin_all.mean()
    m2 = 1.0 + colmin.mean()
    return np.asarray(np.float32(max(m1, m2)))


def kernel(input_img, target_img, inds_y_input, inds_x_input,
           inds_y_target, inds_x_target):
    global _compiled
    import time

    from concourse import bass_utils

    if _compiled is None:
        _compiled = _build_v2()
    nc = _compiled

    in_maps = _prepare_inputs_v2(input_img, target_img, inds_y_input,
                                 inds_x_input, inds_y_target, inds_x_target)
    # Retry: a previously-crashed tenant can leave the NeuronCore wedged
    # (NRT_EXEC_UNIT_UNRECOVERABLE) for one execution attempt before it
    # self-clears; a fresh attempt then succeeds.
    last_err = None
    for attempt in range(4):
        try:
            res = bass_utils.run_bass_kernel_spmd(
                nc, in_maps, core_ids=list(range(NCORES)))
            return _combine_v2(res.results)
        except Exception as e:  # noqa: BLE001
            last_err = e
            time.sleep(3.0)
    raise last_err

